# revision 30
# baseline (speedup 1.0000x reference)
"""Trainium2 Bass kernel for nn_LHFA_76278619177511.

Fused transposed-attention block (LHFA):
  q = dwconv3(conv1x1(x, Wq), Wq_dw)   (same for k from y, v from z)
  attn = softmax(l2norm(q) @ l2norm(k)^T * temp)   per-head [32,32]
  out = Wproj @ (attn @ v) + Wfus @ [x;y;z]

Strategy (per core, pure data-parallel over batch B=8 on 8 cores):
  - Every matmul runs in fp8e4m3 with MatmulPerfMode.DoubleRow (K=256
    per instruction, 0.5 PE cycles per streamed column). The fusion
    conv keeps precision via an exact hi/lo fp8 split of both acts and
    weights (3 DR matmuls: hiWh + loWh + hiWl), since its output
    dominates |out|. q/k/v scales cancel in the l2 normalization or
    are folded into the output descale (PSUM holds 2048*out).
  - The depthwise 3x3 folds into the 1x1 conv: merged weights with
    K=576 contracted in 3 DoubleRow steps over a host-packed [A;B]
    pad tile (B = A shifted one image ROW, pitch 256), so each j-pair
    at stride 512 covers a full dw-column of the stencil.
  - q,k are produced directly TRANSPOSED ([n,c]); per-head Gram
    matrices q@k^T, q@q^T, k@k^T accumulate 2 image rows per DR
    matmul; v in natural layout [c,n].
  - Wproj is folded into the tiny block-diag attn on-chip (W2 =
    Wproj@attnBD), so pass 2 is a single matmul stage over v plus the
    fusion terms -- no attn@v intermediate or its PSUM roundtrip.
  - Row norms from gram diagonals; softmax on [256,32] blocks runs on
    DVE (broadcast muls + 32x32 block transposes) with only exp on
    Act; act tables are prefetched off the critical path.
  - Host packs pad tiles and fusion tiles into DMA-friendly streams
    (one big contiguous transfer each); all DMA is issued from SP/Pool
    so Act/DVE stay free for the mandatory PSUM->SBUF copies.
"""

import numpy as np
import ml_dtypes

import bass_rust
import concourse.bass as bass
import concourse.mybir as mybir
from concourse import tile as tile_mod
from concourse.tile import TileContext
from concourse.vector_clock import ScopedClock
from concourse.bass_utils import run_bass_kernel_spmd

BF16 = mybir.dt.bfloat16
F16 = mybir.dt.float16
F32 = mybir.dt.float32
FP8 = mybir.dt.float8e4
DR = mybir.MatmulPerfMode.DoubleRow

C = 64          # input channels
DIM = 256       # q/k/v channels
HEADS = 8
H = W = 128
N = H * W       # 16384
PITCH = 256     # pad-tile row pitch (Ldweights k-subtile strides must be
                # 128/256/512, so consecutive rows sit exactly 256 apart)
HB = 16         # band height (output rows per band)
NB = H // HB    # 8 bands
TROWS = HB + 3  # 18 data rows + 1 zeroed spare for junk k-subtiles
TW = TROWS * PITCH  # 4864 cols per pad tile

W_SCALE = 1024.0    # merged conv weight scale (cancels for q/k)
V_DESCALE = 16.0    # v PSUM->SBUF copy scale 1/16 -> v_slab holds 64*v
BD_SCALE = 8.0      # attn block-diag scale
WP_SCALE = 32.0     # Wproj scale
WFOLD_DESCALE = 8.0  # wfold copy scale 1/8 -> wfold holds 32*(Wproj@attn)
OUT_SCALE = (W_SCALE / V_DESCALE) * (BD_SCALE * WP_SCALE / WFOLD_DESCALE)
# = 64 * 32 = 2048: PSUM holds 2048*out; fp8 Wfus_s = 2048*Wfus stays in range

_PATCHED = False


def _patch_tile_drain():
    """This walrus build rejects >1 sem wait on a CTRL (Drain) instruction;
    split the TileContext tail-drain waits onto individual nops."""
    global _PATCHED
    if _PATCHED:
        return
    _PATCHED = True

    def _drain_and_barrier(self, tick_clock, wait_clock):
        nc = self.nc
        drain_inst = nc.sync.drain()
        wait_clock.add_sem_waits(
            drain_inst.ins, ScopedClock({None: tick_clock.global_clock})
        )
        si = drain_inst.ins.sync_info
        waits = list(si.on_wait or [])
        if len(waits) > 1:
            si.on_wait = waits[:1]
            for w in waits[1:]:
                nop = nc.sync.nop(nofuse=True, hint="split_wait")
                nop.ins.sync_info = bass_rust.SyncInfo(on_wait=[w], on_update=[])
        nc.all_engine_barrier()
        assert self.sems is not None
        popped = nc._tile_sem_poison_stack.pop()
        assert popped is self._sem_poison
        nc.clear_and_free_semaphores(list(self.sems.allocated().values()))
        nc.all_engine_barrier()

    tile_mod.TileContext._drain_and_barrier = _drain_and_barrier
    try:
        from concourse import tile_utils
        tile_utils.max_sbuf_usage = 208 * 1024
    except Exception:
        pass


def _split_excess_waits(nc, max_waits=1):
    """This walrus build caps sem waits per instruction encoding; hoist
    excess waits onto preceding same-engine NoOps (queues are in-order,
    so a wait satisfied on an earlier instruction orders the later one)."""
    import bass_rust as _br

    ctr = [0]
    for f in nc.m.functions:
        for blk in f.blocks:
            out = []
            for inst in blk.instructions:
                si = inst.sync_info
                waits = list(si.on_wait) if (si and si.on_wait) else []
                if len(waits) > max_waits:
                    keep = waits[:max_waits]
                    extra = waits[max_waits:]
                    si.on_wait = keep
                    for w in extra:
                        ctr[0] += 1
                        nop = _br.InstNoOp(name=f"wsplit-{ctr[0]}", ins=[], outs=[])
                        nop.engine = inst.engine
                        nop.sync_info = _br.SyncInfo(on_wait=[w], on_update=[])
                        try:
                            nc.register_instruction(nop, overwrite=True)
                        except Exception:
                            pass
                        out.append(nop)
                out.append(inst)
            blk.instructions[:] = out


# B half = image shifted one ROW vs A, so a DoubleRow pair at j-stride 512
# (2 tile rows) covers a full dw-column of the 3x3 stencil:
#   pair p (dw col p): j0 -> A tap (0,p) + B tap (1,p);
#                      j1 -> A tap (2,p) + B spill (zero weight, junk row).
# Groups g = 2p + j, each (A-half tap, B-half tap-or-None):
TAPS = [
    ((0, 0), (1, 0)), ((2, 0), None),
    ((0, 1), (1, 1)), ((2, 1), None),
    ((0, 2), (1, 2)), ((2, 2), None),
]


def _merge_w(W1, Wdw):
    """-> [128, 6, 256] f32 (scaled)."""
    out = np.zeros((128, 6, 256), np.float32)
    W1 = W1[:, :, 0, 0]  # [256, 64]
    for g, (ta, tb) in enumerate(TAPS):
        out[0:64, g, :] = (Wdw[:, 0, ta[0], ta[1]][:, None] * W1).T
        if tb is not None:
            out[64:128, g, :] = (Wdw[:, 0, tb[0], tb[1]][:, None] * W1).T
    return out * W_SCALE


def _f8(a):
    return np.ascontiguousarray(np.asarray(a)).astype(ml_dtypes.float8_e4m3)


def _pack_pads(img8):
    """[64, H, W] fp8 -> [128, NB*TW] pad-tile stream: per band, 19 rows of
    pitch 256 ([0, img, 0, dead]); A half rows = img HB*b-1+t, B = HB*b+t."""
    arr = np.zeros((128, NB, TROWS, PITCH), ml_dtypes.float8_e4m3)
    for b in range(NB):
        for t in range(HB + 2):
            rA = HB * b - 1 + t
            if 0 <= rA < H:
                arr[0:64, b, t, 1:129] = img8[:, rA, :]
            rB = HB * b + t
            if rB < H:
                arr[64:128, b, t, 1:129] = img8[:, rB, :]
    return np.ascontiguousarray(arr.reshape(128, NB * TW))


def _pack_f(xy, zz):
    """xy [128,N] f32, zz [64,N] f32 -> [128, 32*1536] fp8 hi/lo tiles."""
    xy_hi = xy.astype(ml_dtypes.float8_e4m3)
    z_hi = zz.astype(ml_dtypes.float8_e4m3)
    xy_lo = (xy - xy_hi.astype(np.float32)).astype(ml_dtypes.float8_e4m3)
    z_lo = (zz - z_hi.astype(np.float32)).astype(ml_dtypes.float8_e4m3)
    arr = np.zeros((128, 32, 3, 512), ml_dtypes.float8_e4m3)
    arr[:, :, 0, :] = xy_hi.reshape(128, 32, 512)
    arr[:, :, 1, :] = xy_lo.reshape(128, 32, 512)
    arr[0:64, :, 2, :] = z_hi.reshape(64, 32, 512)
    arr[64:128, :, 2, :] = z_lo.reshape(64, 32, 512)
    return np.ascontiguousarray(arr.reshape(128, 32 * 1536)), xy_hi, z_hi


def _dr_pair(ap2d, j_stride, m_count):
    """[128, X] slice -> [128, j(2, j_stride), m(m_count, 1)] overlapping AP."""
    a = ap2d.unsqueeze(1)
    ap = a.ap
    ap[1] = [j_stride, 2]
    ap[2] = [1, m_count]
    return a


def _build_nc(wq, wk, wv, wprojT, wfus8, temp_cols):
    """Build the Bass module. wq/wk/wv: [128,6,256] f32 merged+scaled;
    wprojT [128,512] f32 (x WP_SCALE); wfus8 [128,1536] fp8 hi/lo groups."""
    _patch_tile_drain()
    nc = bass.Bass()

    # host-packed pad tiles ([A;B] halves, zeros baked in) and pass-2
    # fusion tiles -- one big contiguous DMA each, no memsets
    xp8d = nc.declare_dram_parameter("xp8", [128, NB * TW], FP8, isOutput=False)
    yp8d = nc.declare_dram_parameter("yp8", [128, NB * TW], FP8, isOutput=False)
    zp8d = nc.declare_dram_parameter("zp8", [128, NB * TW], FP8, isOutput=False)
    fpkd = nc.declare_dram_parameter("fpk8", [128, 32 * 1536], FP8, isOutput=False)
    od = nc.declare_dram_parameter("out", [DIM, N], F16, isOutput=True)

    wq_d = nc.inline_tensor(_f8(wq.reshape(128, 6 * 256)), name="wq9")
    wk_d = nc.inline_tensor(_f8(wk.reshape(128, 6 * 256)), name="wk9")
    wv_d = nc.inline_tensor(_f8(wv.reshape(128, 6 * 256)), name="wv9")
    wp_d = nc.inline_tensor(_f8(wprojT), name="wprojT")   # [128, 512]
    wf_d = nc.inline_tensor(wfus8, name="wfus8")          # [128, 1536] fp8
    tc0_d = nc.inline_tensor(np.ascontiguousarray(temp_cols[0]), name="tcol0")
    eye = np.eye(128, dtype=ml_dtypes.bfloat16)
    id_d = nc.inline_tensor(
        np.ascontiguousarray(np.concatenate([eye, eye], axis=1)), name="ident"
    )
    tc1_d = nc.inline_tensor(np.ascontiguousarray(temp_cols[1]), name="tcol1")

    with TileContext(nc) as tc:
        import contextlib

        with contextlib.ExitStack() as ctx:
            wpool = ctx.enter_context(tc.tile_pool(name="wpool", bufs=1))
            vpool = ctx.enter_context(tc.tile_pool(name="vpool", bufs=1))
            pads = ctx.enter_context(tc.tile_pool(name="pads", bufs=2))
            qkp = ctx.enter_context(tc.tile_pool(name="qkp", bufs=4))
            smallp = ctx.enter_context(tc.tile_pool(name="smallp", bufs=2))
            p2p = ctx.enter_context(tc.tile_pool(name="p2p", bufs=5))

            # --- weights to SBUF ---
            wq_sb = wpool.tile([128, 6 * 256], FP8, tag="wq")
            wk_sb = wpool.tile([128, 6 * 256], FP8, tag="wk")
            wv_sb = wpool.tile([128, 6 * 256], FP8, tag="wv")
            wp_sb = wpool.tile([128, 512], FP8, tag="wp")
            wf_sb = wpool.tile([128, 1536], FP8, tag="wf")
            wfold_sb = wpool.tile([128, 512], FP8, tag="wfold")
            ident_sb = wpool.tile([128, 256], BF16, tag="ident")
            tcol = [wpool.tile([128, 1], F32, tag=f"tc{i}", name=f"tcol{i}") for i in range(2)]
            # Act is idle until the first conv results land ~3us in; use it
            # for the startup weight loads so SP/Pool start on band-0 tiles
            # wk first: the first conv matmul of band 0 consumes k (from y)
            nc.scalar.dma_start(out=wk_sb, in_=wk_d[:])
            nc.scalar.dma_start(out=wq_sb, in_=wq_d[:])
            nc.scalar.dma_start(out=wv_sb, in_=wv_d[:])
            # warm Ln then Exp at kernel start: both live in the
            # natural_log_exp set (as does Copy), so the whole kernel needs
            # exactly ONE act-table load, here, off the critical path. Norms
            # use 1/sqrt(s) = exp(-0.5*ln(s)) instead of Sqrt (whose table
            # set does not contain Exp and would force two ~1.3us reloads
            # inside the softmax chain).
            warm = smallp.tile([128, 1], F32, tag="warm", name="warm")
            nc.vector.memset(warm, 1.0)
            nc.scalar.activation(
                warm, warm, mybir.ActivationFunctionType.Ln, bias=0.0, scale=1.0
            )
            nc.scalar.activation(
                warm, warm, mybir.ActivationFunctionType.Exp, bias=0.0, scale=1.0
            )


            # --- persistent state ---
            v_slab = vpool.tile([128, 2 * N], FP8, tag="vslab", name="vslab")
            ps_acc = ctx.enter_context(tc.tile_pool(name="ps_acc", bufs=1, space="PSUM"))
            pqv_stack = ctx.enter_context(contextlib.ExitStack())
            ps_qk = pqv_stack.enter_context(tc.tile_pool(name="ps_qk", bufs=2, space="PSUM"))
            ps_v = pqv_stack.enter_context(tc.tile_pool(name="ps_v", bufs=2, space="PSUM"))
            acc1 = ps_acc.tile([128, 512], F32, tag="acc1")
            acc2 = ps_acc.tile([128, 256], F32, tag="acc2")
            par_all = acc1[:, 0:256]
            pgq = acc1[:, 256:512]
            pgk = acc2


            # only Act/DVE may read PSUM; balance them by modeled cost
            cp_load = [0.0, 0.0]  # ns spent: [Act, DVE]

            def big_copy(dst, src, scale=None, force=None):
                n = 1
                for _, ct in src.ap[1:]:
                    n *= ct
                costs = ((n + 172) * 0.833, (n + 120) * 1.042)
                if force is None:
                    i = 0 if cp_load[0] + costs[0] <= cp_load[1] + costs[1] else 1
                else:
                    i = force
                cp_load[i] += costs[i]
                if i == 0:
                    if scale is None:
                        nc.scalar.copy(dst, src)
                    else:
                        nc.scalar.activation(
                            dst, src, mybir.ActivationFunctionType.Copy,
                            bias=0.0, scale=scale,
                        )
                else:
                    if scale is None:
                        nc.vector.tensor_copy(dst, src)
                    else:
                        nc.vector.tensor_scalar_mul(dst, src, scale)

            def dr_rhs_w(w_sb, r):  # weights moving [128, j(256), 256]
                return w_sb[:, r * 512:(r + 1) * 512].rearrange(
                    "p (j n) -> p j n", j=2
                )

            def dr_lhsT_w(w_sb, r, mb):  # weights stationary [128, j(256), 128]
                return _dr_pair(
                    w_sb[:, r * 512 + mb * 128: r * 512 + mb * 128 + 384], 256, 128
                )

            def dr_act(AB, row, p):  # conv activations [128, j(512), 128]
                off = row * PITCH + p
                return _dr_pair(AB[:, off: off + 640], 512, 128)


            def gram_ap(cat2, off):  # [128, j(512), 128]
                return _dr_pair(cat2[:, off: off + 640], 512, 128)

            def do_grams(cat2, first, last):
                for mb in range(2):
                    qs = gram_ap(cat2, 256 + mb * 128)
                    ks = gram_ap(cat2, mb * 128)
                    for dst, lh, rh in (
                        (par_all[:, mb * 128: mb * 128 + 128], qs, ks),
                        (pgq[:, mb * 128: mb * 128 + 128], qs, qs),
                        (pgk[:, mb * 128: mb * 128 + 128], ks, ks),
                    ):
                        nc.tensor.matmul(
                            dst, lhsT=lh, rhs=rh, start=first, stop=last,
                            skip_group_check=True, perf_mode=DR,
                        )

            def do_v_units(b, zAB, units, forces=None):
                for ui, (cc, mb) in enumerate(units):
                    pv_t = ps_v.tile([128, 512], F32, tag="pv")
                    for rr in range(4):
                        for r in range(3):
                            nc.tensor.matmul(
                                pv_t[:, rr * 128:(rr + 1) * 128],
                                lhsT=dr_lhsT_w(wv_sb, r, mb),
                                rhs=dr_act(zAB, 4 * cc + rr, r),
                                start=(r == 0),
                                stop=(r == 2),
                                perf_mode=DR,
                            )
                    big_copy(
                        v_slab[:, mb * N + (HB * b + 4 * cc) * W:
                               mb * N + (HB * b + 4 * cc) * W + 512],
                        pv_t,
                        scale=1.0 / V_DESCALE,
                        # band 7: keep Act clear for the softmax-critical
                        # cat copies / exp chain
                        force=(forces[ui] if forces is not None
                               else (1 if b == NB - 1 else None)),
                    )

            ins_d = [xp8d, yp8d, zp8d]

            # ================= pass 1: bands =================
            for b in range(NB):
                if b == 1:
                    # softmax-chain constants: issued after band-0's critical
                    # loads, resident long before the chain needs them
                    nc.gpsimd.dma_start(out=ident_sb, in_=id_d[:])
                    nc.gpsimd.dma_start(out=tcol[0], in_=tc0_d[:])
                    nc.gpsimd.dma_start(out=tcol[1], in_=tc1_d[:])
                srcs = []
                # DMA blocks the issuing engine in the cost model; keep all
                # DMA on SP/Pool so Act/DVE stay free for PSUM copies.
                load_engs = (
                    nc.sync, nc.gpsimd,
                    nc.sync if b % 2 else nc.gpsimd,
                )
                for ti, td in enumerate(ins_d):
                    nm = "xyz"[ti]
                    AB = pads.tile([128, TW], FP8, tag=f"{nm}AB")

                    if b == 0 and ti < 2:
                        # conveyor of small chunks: pair p needs tile rows
                        # <= 2p+5, so rows arrive just ahead of the PE
                        cuts = [0, 4, 8, 12, TROWS]
                        for c0, c1 in zip(cuts, cuts[1:]):
                            load_engs[ti].dma_start(
                                out=AB[:, c0 * PITCH: c1 * PITCH],
                                in_=td[:, c0 * PITCH: c1 * PITCH],
                            )
                    else:
                        load_engs[ti].dma_start(
                            out=AB, in_=td[:, b * TW:(b + 1) * TW]
                        )
                    srcs.append(AB)
                # band 0: zAB lands last (serialized behind xAB on SP), so
                # push ALL its v-conv chunks past the grams (z arrives ~5.4us,
                # the deferred units start ~8.8us). Band 7: defer half the v
                # chunks past the final grams so the PE chews on them while
                # the softmax chain (which only needs the grams) runs.
                if b == 0:
                    v_units = [[] for _ in range(9)]
                elif b == NB - 1:
                    v_units = [[divmod(ph, 2)] for ph in range(4)] + [[]] * 5
                else:
                    v_units = [[divmod(ph, 2)] for ph in range(8)] + [[]]

                cat_pairs = []
                for ph in range(HB // 2):
                    cat2 = qkp.tile([128, 1024], FP8, tag="cat")
                    cat_pairs.append(cat2)
                    pqk2 = ps_qk.tile([128, 1024], F32, tag="pqk")
                    for sub in range(2):
                        hl = 2 * ph + sub
                        for src_i, w_sb, col0 in (
                            (1, wk_sb, 0), (0, wq_sb, 256)
                        ):
                            AB = srcs[src_i]
                            for r in range(3):
                                nc.tensor.matmul(
                                    pqk2[:, sub * 512 + col0:
                                         sub * 512 + col0 + 256],
                                    lhsT=dr_act(AB, hl, r),
                                    rhs=dr_rhs_w(w_sb, r),
                                    start=(r == 0),
                                    stop=(r == 2),
                                    perf_mode=DR,
                                )
                    if b == NB - 1 and ph >= 5:
                        # band-7 tail: halve the cat-copy latency by running
                        # the two halves on Act and DVE in parallel, so the
                        # final grams (and the softmax chain behind them)
                        # start ~1us earlier
                        big_copy(cat2[:, 0:512], pqk2[:, 0:512], force=0)
                        big_copy(cat2[:, 512:1024], pqk2[:, 512:1024], force=1)
                    else:
                        big_copy(cat2, pqk2)

                    # one v-conv half-chunk per pair (8 per band; band 0
                    # defers them until zAB has landed)
                    do_v_units(b, srcs[2], v_units[ph])

                    if ph > 0:
                        pr = (HB // 2) * b + ph - 1
                        do_grams(cat_pairs[ph - 1], pr == 0, False)
                pr = (HB // 2) * b + (HB // 2) - 1
                do_grams(cat_pairs[-1], pr == 0, pr == (H // 2) - 1)
                if b == 0:
                    do_v_units(b, srcs[2],
                               [(cc, mb) for cc in range(4) for mb in range(2)])
                elif b == NB - 1:
                    # deferred half-band of v MATMULS fill the PE while the
                    # softmax chain runs. Homes: 2 ps_v tiles + one ps_qk
                    # pair tile, so no matmul waits on a PSUM copy; the
                    # copies are issued later (mid-chain / post-chain) so
                    # they trail the softmax ops in the Act/DVE queues.
                    defer_pqa = ps_qk.tile([128, 1024], F32, tag="pqk")
                    defer_pqb = ps_qk.tile([128, 1024], F32, tag="pqk")
                    homes = [(defer_pqa, 0), (defer_pqa, 512),
                             (defer_pqb, 0), (defer_pqb, 512)]
                    for ui, (cc, mb) in enumerate(
                            [(2, 0), (2, 1), (3, 0), (3, 1)]):
                        t, off = homes[ui]
                        for rr in range(4):
                            for r in range(3):
                                nc.tensor.matmul(
                                    t[:, off + rr * 128: off + rr * 128 + 128],
                                    lhsT=dr_lhsT_w(wv_sb, r, mb),
                                    rhs=dr_act(srcs[2], 4 * cc + rr, r),
                                    start=(r == 0),
                                    stop=(r == 2),
                                    perf_mode=DR,
                                )

            nc.sync.dma_start(out=wp_sb, in_=wp_d[:])
            nc.sync.dma_start(out=wf_sb, in_=wf_d[:])

            # ========= phase 1.5: softmax on [256, 32] =========
            # Issued BEFORE the PSUM pool close so (a) the softmax ops lead
            # the deferred-v drains in the Act/DVE queues, and (b) every
            # deferred-tile release precedes pass-2 bank reuse (the Tile
            # allocator requires releases to be issued before aliasing
            # writers). The Wproj-fold matmuls are issued AFTER the pass-2
            # fusion pre-issue so the in-order PE queue never blocks on bd2
            # before reaching that independent fill work.
            ar_sb = [smallp.tile([128, 128], F32, tag=f"arsb{mb}", name=f"arsb{mb}") for mb in range(2)]
            nc.scalar.copy(ar_sb[0], par_all[:, 0:128])
            nc.scalar.copy(ar_sb[1], par_all[:, 128:256])
            # bd2[c_local, mb*128 + d] = attn[c, d] * BD_SCALE, block-diag
            bd2 = smallp.tile([128, 256], FP8, tag="bd2", name="bdiag")
            nc.gpsimd.memset(bd2, 0.0)
            # norms: masked mul + 3D reduce on DVE (one pair of ops per
            # gram), then ONE batched ln + exp(-0.5*x) on Act gives all four
            # 1/sqrt(ssum) columns with no table switch (ln/exp/copy share
            # the natural_log_exp table loaded at kernel start). The
            # max(.,1e-12) of the reference never binds: |q_s| row norms
            # are ~1e3 by construction.
            ssum4 = smallp.tile([128, 4], F32, tag="ssum4", name="ssum4")
            rn4 = smallp.tile([128, 4], F32, tag="rn4", name="rn4")
            scr = smallp.tile([128, 256], F32, tag="scr")
            for gi, g_ps in enumerate((pgq, pgk)):
                # one masked mul + one 3D reduce covers both mb blocks
                # (ident_sb is [I|I]); ssum4 layout: [q0, q1, k0, k1]
                nc.vector.tensor_mul(scr, g_ps, ident_sb)
                nc.vector.reduce_sum(
                    out=ssum4[:, 2 * gi: 2 * gi + 2],
                    in_=scr.rearrange("p (j n) -> p j n", j=2),
                    axis=mybir.AxisListType.X,
                )
            nc.scalar.activation(
                rn4, ssum4, mybir.ActivationFunctionType.Ln, bias=0.0, scale=1.0
            )
            nc.scalar.activation(
                rn4, rn4, mybir.ActivationFunctionType.Exp, bias=0.0, scale=-0.5
            )
            rn = [(rn4[:, 0:1], rn4[:, 2:3]), (rn4[:, 1:2], rn4[:, 3:4])]
            # the two mb chains run on DIFFERENT engines so they advance in
            # parallel: mb0 on DVE (broadcast muls), mb1 on Act (activation
            # per-partition scale); only transposes/reduce/recip of mb1 and
            # the exps cross engines.
            COPY = mybir.ActivationFunctionType.Copy
            for mb in range(2):
                rnq_c, rnk_c = rn[mb]
                rnqt = smallp.tile([128, 1], F32, tag=f"rnqt{mb}", name=f"rnqt{mb}")
                hd = smallp.tile([128, 32], F32, tag=f"hd{mb}", name=f"hd{mb}")
                if mb == 0:
                    nc.vector.tensor_mul(rnqt, rnq_c, tcol[mb])
                    for i in range(4):
                        nc.vector.tensor_mul(
                            hd[32 * i: 32 * (i + 1), :],
                            ar_sb[mb][32 * i: 32 * (i + 1), bass.ds(32 * i, 32)],
                            rnqt[32 * i: 32 * (i + 1), :].to_broadcast([32, 32]),
                        )
                else:
                    nc.scalar.activation(rnqt, tcol[mb], COPY, bias=0.0, scale=rnq_c)
                    for i in range(4):
                        nc.scalar.activation(
                            hd[32 * i: 32 * (i + 1), :],
                            ar_sb[mb][32 * i: 32 * (i + 1), bass.ds(32 * i, 32)],
                            COPY, bias=0.0,
                            scale=rnqt[32 * i: 32 * (i + 1), :],
                        )
                hdT = smallp.tile([128, 32], F32, tag=f"hdT{mb}")
                nc.vector.transpose(hdT, hd)
                hdTs = smallp.tile([128, 32], F32, tag=f"hdTs{mb}")
                if mb == 0:
                    nc.vector.tensor_mul(hdTs, hdT, rnk_c.to_broadcast([128, 32]))
                else:
                    nc.scalar.activation(hdTs, hdT, COPY, bias=0.0, scale=rnk_c)
                hd3 = smallp.tile([128, 32], F32, tag=f"hd3{mb}")
                nc.vector.transpose(hd3, hdTs)
                # no max-subtraction: logits are cosines * temp, |x| <= ~1,
                # so exp cannot overflow and softmax is algebraically equal
                ex = smallp.tile([128, 32], F32, tag=f"ex{mb}")
                rsm = smallp.tile([128, 1], F32, tag=f"rsm{mb}")
                # accum_out gives the row sum for free: kills the DVE
                # reduce and one cross-engine hop on the critical chain
                nc.scalar.activation(ex, hd3, mybir.ActivationFunctionType.Exp,
                                     bias=0.0, scale=1.0, accum_out=rsm)
                nc.vector.reciprocal(rsm, rsm)
                nc.vector.tensor_scalar_mul(rsm, rsm, BD_SCALE)
                for i in range(4):
                    if mb == 0:
                        nc.vector.tensor_mul(
                            bd2[32 * i: 32 * (i + 1), 32 * i: 32 * i + 32],
                            ex[32 * i: 32 * (i + 1), :],
                            rsm[32 * i: 32 * (i + 1), :].to_broadcast([32, 32]),
                        )
                    else:
                        nc.scalar.activation(
                            bd2[32 * i: 32 * (i + 1),
                                128 + 32 * i: 128 + 32 * i + 32],
                            ex[32 * i: 32 * (i + 1), :],
                            COPY, bias=0.0,
                            scale=rsm[32 * i: 32 * (i + 1), :],
                        )
                # fold Wproj into this mb's attn immediately, overlapping
                # the other mb's chain (acc1 reuse: par/pgq fully consumed)
                pw2 = acc1[:, mb * 256:(mb + 1) * 256]
                nc.tensor.matmul(
                    pw2,
                    lhsT=bd2[:, mb * 128:(mb + 1) * 128],
                    rhs=wp_sb[:, mb * 256:(mb + 1) * 256],
                    start=True,
                    stop=True,
                )
                # mb0 copy on DVE, mb1 on Act
                big_copy(wfold_sb[:, mb * 256:(mb + 1) * 256], pw2,
                         scale=1.0 / WFOLD_DESCALE, force=1 - mb)

            # merged strided drains of the deferred pair tiles. The drains
            # became READY as soon as the deferred matmuls finished, and the
            # readiness-greedy Tile scheduler then ran these 1us copies right
            # at the head of the softmax chain on both engines. Gate each
            # drain on a token derived from the wfold copies so it cannot be
            # scheduled before the chain (and its wfold copies) retire.
            tokA = smallp.tile([128, 1], F32, tag="tokA", name="tokA")
            tokB = smallp.tile([128, 1], F32, tag="tokB", name="tokB")
            nc.scalar.activation(tokA, wfold_sb[:, 256:257], COPY,
                                 bias=1.0 / V_DESCALE, scale=0.0)
            nc.vector.tensor_scalar(tokB, wfold_sb[:, 0:1], 0.0,
                                    1.0 / V_DESCALE, mybir.AluOpType.mult,
                                    mybir.AluOpType.add)
            for pq_t, r0, tok, f in ((defer_pqa, 120, tokA, 0),
                                     (defer_pqb, 124, tokB, 1)):
                vdst = v_slab[:, r0 * W: r0 * W + 512].unsqueeze(1)
                vap = vdst.ap
                vap[1] = [N, 2]
                vap[2] = [1, 512]
                src = pq_t.rearrange("p (j n) -> p j n", j=2)
                if f == 0:
                    nc.scalar.activation(vdst, src, COPY, bias=0.0, scale=tok)
                else:
                    nc.vector.tensor_scalar_mul(vdst, src, tok)

            # free the conv PSUM banks and start pass-2 fusion work now so
            # the PE has something to chew on during the softmax transition
            pqv_stack.close()
            ps_po = ctx.enter_context(tc.tile_pool(name="ps_po", bufs=3, space="PSUM"))

            def p2_load(ch):
                # F: [xy_hi | xy_lo | z_hi(top);z_lo(bottom)], host-packed
                f_t = p2p.tile([128, 1536], FP8, tag="ft", name="f_t")
                (nc.sync if ch % 2 else nc.gpsimd).dma_start(
                    out=f_t, in_=fpkd[:, ch * 1536:(ch + 1) * 1536]
                )
                return f_t

            def p2_fus(ch, f_t):
                po2 = ps_po.tile([128, 1024], F32, tag="po", name="po")
                for mb in range(2):
                    po = po2[:, mb * 512:(mb + 1) * 512]
                    for i, jstride in ((0, 512), (1, 1024), (2, 1024)):
                        fa = f_t[:, 0:512].unsqueeze(1)
                        ap = fa.ap
                        ap[1] = [jstride, 2]
                        ap[2] = [1, 512]
                        nc.tensor.matmul(
                            po,
                            lhsT=_dr_pair(
                                wf_sb[:, i * 512 + mb * 128:
                                      i * 512 + mb * 128 + 384],
                                256, 128,
                            ),
                            rhs=fa,
                            start=(i == 0),
                            stop=False,
                            perf_mode=DR,
                        )
                return po2

            def p2_wfold(ch, po2):
                n0 = 512 * ch
                for mb in range(2):
                    vs = v_slab[:, n0: n0 + 512].unsqueeze(1)
                    ap = vs.ap
                    ap[1] = [N, 2]
                    ap[2] = [1, 512]
                    nc.tensor.matmul(
                        po2[:, mb * 512:(mb + 1) * 512],
                        lhsT=_dr_pair(wfold_sb[:, mb * 128: mb * 128 + 384], 256, 128),
                        rhs=vs,
                        start=False,
                        stop=True,
                        perf_mode=DR,
                    )
                o_t = p2p.tile([128, 1024], F16, tag="ot", name="o_t")
                # out[mb*128+p, n0+c] <- o_t[p, mb*512+c]
                dst = od[:].rearrange("(m p) n -> p m n", m=2)[:, :, n0: n0 + 512]
                o_tv = o_t.rearrange("p (m c) -> p m c", m=2)
                if ch >= 30:
                    # drain tail: split copy + store across both engines /
                    # both DMA queues so the pipeline flush is ~2x shorter
                    nc.scalar.activation(
                        o_t[:, 0:512], po2[:, 0:512],
                        mybir.ActivationFunctionType.Copy,
                        bias=0.0, scale=1.0 / OUT_SCALE,
                    )
                    nc.vector.tensor_scalar_mul(
                        o_t[:, 512:1024], po2[:, 512:1024], 1.0 / OUT_SCALE
                    )
                    nc.sync.dma_start(out=dst[:, 0:1, :], in_=o_tv[:, 0:1, :])
                    nc.gpsimd.dma_start(out=dst[:, 1:2, :], in_=o_tv[:, 1:2, :])
                else:
                    # pin ch29 to Act so DVE is idle when ch30/31 finish
                    big_copy(o_t, po2, scale=1.0 / OUT_SCALE,
                             force=0 if ch == 29 else None)
                    (nc.sync if ch % 2 == 0 else nc.gpsimd).dma_start(
                        out=dst, in_=o_tv
                    )

            loaded = [p2_load(c) for c in range(4)]
            fus_pend = [p2_fus(c, loaded[c]) for c in range(3)]

            # ============ pass 2: (Wproj@attn) @ v + fusion ============
            # Fold Wproj into the tiny block-diag attn once:
            #   W2 = Wproj_s @ attnBD_s   (dense [256,256], fp8 in SBUF)
            # then out = W2 @ v + Wfus @ [x;y;z] in a single matmul stage --
            # no attn@v intermediate, no PSUM->SBUF roundtrip for it.
            for ch in range(3):
                p2_wfold(ch, fus_pend[ch])
            for ch in range(3, 32):
                if ch + 1 < 32:
                    loaded.append(p2_load(ch + 1))
                p2_wfold(ch, p2_fus(ch, loaded[ch]))

    _split_excess_waits(nc)
    return nc


def _prep(inputs):
    """Host-side weight prep -> args for _build_nc."""
    wq = _merge_w(np.asarray(inputs["Wq"], np.float32), np.asarray(inputs["Wq_dw"], np.float32))
    wk = _merge_w(np.asarray(inputs["Wk"], np.float32), np.asarray(inputs["Wk_dw"], np.float32))
    wv = _merge_w(np.asarray(inputs["Wv"], np.float32), np.asarray(inputs["Wv_dw"], np.float32))

    wproj = np.asarray(inputs["Wproj"], np.float32)[:, :, 0, 0]  # [256,256] out,in
    wprojT = np.zeros((128, 512), np.float32)
    for kb in range(2):
        # [p, kb*256 + m] = Wproj[m, kb*128 + p]
        wprojT[:, kb * 256:(kb + 1) * 256] = wproj[:, kb * 128:(kb + 1) * 128].T
    wprojT *= WP_SCALE

    # fusion weights: hi/lo fp8 split of OUT_SCALE * Wfus.
    # groups (x256 cols, mb*128 sub-blocks):
    #   G0=G1=Wh_xy, G2=Wl_xy, G3=[Wh_z;Wh_z], G4=0, G5=[Wl_z;0]
    wfus = np.asarray(inputs["Wfus"], np.float32)[:, :, 0, 0] * OUT_SCALE
    wh = wfus.astype(ml_dtypes.float8_e4m3).astype(np.float32)
    wl = wfus - wh
    wfus8 = np.zeros((128, 6, 256), np.float32)
    wfus8[:, 0, :] = wh[:, 0:128].T
    wfus8[:, 1, :] = wh[:, 0:128].T
    wfus8[:, 2, :] = wl[:, 0:128].T
    wfus8[0:64, 3, :] = wh[:, 128:192].T
    wfus8[64:128, 3, :] = wh[:, 128:192].T
    wfus8[0:64, 5, :] = wl[:, 128:192].T
    wfus8 = np.ascontiguousarray(
        wfus8.reshape(128, 1536).astype(ml_dtypes.float8_e4m3)
    )

    temp = np.asarray(inputs["temperature"], np.float32).reshape(HEADS)
    tfull = np.repeat(temp, 32).astype(np.float32)
    temp_cols = [tfull[0:128].reshape(128, 1), tfull[128:256].reshape(128, 1)]
    return wq, wk, wv, wprojT, wfus8, temp_cols


def kernel(**inputs):
    x = np.asarray(inputs["x"], np.float32)
    y = np.asarray(inputs["y"], np.float32)
    z = np.asarray(inputs["z"], np.float32)
    B = x.shape[0]
    assert B == 8

    nc = _build_nc(*_prep(inputs))

    in_maps = []
    for i in range(B):
        xy = np.ascontiguousarray(
            np.concatenate([x[i].reshape(C, N), y[i].reshape(C, N)])
        )
        zz = np.ascontiguousarray(z[i].reshape(C, N))
        fpk, xy_hi, z_hi = _pack_f(xy, zz)
        in_maps.append(
            {
                "xp8": _pack_pads(xy_hi[0:64].reshape(C, H, W)),
                "yp8": _pack_pads(xy_hi[64:128].reshape(C, H, W)),
                "zp8": _pack_pads(z_hi.reshape(C, H, W)),
                "fpk8": fpk,
            }
        )
    res = run_bass_kernel_spmd(nc, in_maps, list(range(8)))
    out = np.stack(
        [np.asarray(res.results[i]["out"]).astype(np.float32).reshape(DIM, H, W) for i in range(B)]
    )
    return out

